# revision 17
# baseline (speedup 1.0000x reference)
"""Trainium2 Bass kernel for LlamaMultiheadLatentAttention.

Contract: kernel(**inputs) takes FULL fp32 inputs (as produced by
reference.setup_inputs) and returns the FULL fp32 output [2, 1024, 4096].

Sharding (8 cores, no collectives): core c handles batch b = c//4 and
head-group g = c%4 (8 query heads, 2 kv heads, 8 latent heads). q/k/v and
latent projections are column-sharded per head-group; o_proj/latent_o_proj
are row-sharded, so each core emits a partial output sum and the host adds
the 4 partials per batch (the output all-reduce happens at unshard time).

Key design points (baseline 1.01ms -> ~0.75ms):
  - all matmul operands bf16 (fp8 was measured numerically infeasible for
    the 2e-2 gate: even w_v/w_lv alone in e4m3 gives rel=2.8e-2).
  - w_lq @ w_lk composed on the host into one weight: the lq GEMM
    disappears (-27us TensorE/core) and numerics slightly improve.
  - feature-major activations; attention scores computed transposed
    S^T[j,i] so softmax/PV need no transposes.
  - causal trimming by shortened moving streams: a diagonal j-block's
    score/PV/denominator matmuls only stream queries i >= 128*jb into a
    nested PSUM sub-range (the per-element has_written bits make
    overwrite-then-accumulate across nested ranges correct). -31us PE,
    -25% exp elements, masks shrink to one [128,128] multiply per block.
  - softmax denominator via ones-stationary matmuls -> [1,512]
    reciprocal_approx_fast -> gpsimd partition_broadcast (off the
    DVE/ACT critical path; v1's [1,512] InstReciprocal cost 3.3us each).
  - software-pipelined per-head emission: scores+exp early, next head's
    q-projection between, PV after, so ScalarE's ~110us of exp hides
    entirely under projection matmuls and HAM stays at K=8/8 for the
    whole kernel body.
  - pool/DMA orchestration: weight-stream pool allocated before the B2
    weight pool (new-pool writes wait on the WHOLE previous pool's
    release, which otherwise serializes the first B1 weight DMA behind
    all of B2); warmup-result DMA on the gpsimd queue (on the sync queue
    it head-of-line-blocks every input DMA); startup DMAs chunked and
    first chunks split across queues (one queue sustains ~1/16 of HBM
    BW); HAM warmup matmuls cover the initial DMA latency.
  - all PSUM->SBUF copies pinned to DVE (nc.any copies land on ScalarE
    which is both slow for copies and busy with exp).
"""

import numpy as np
import ml_dtypes

import concourse.bass as bass
import concourse.mybir as mybir
import concourse.tile as tile
from concourse import bacc
from concourse.bass_utils import run_bass_kernel_spmd

BF16 = ml_dtypes.bfloat16

B, S, D = 2, 1024, 4096
H, KVH, HD = 32, 8, 128
GROUPS = H // KVH
LAT, LH = 1024, 32
THETA = 10000.0
SCALE = 1.0 / float(np.sqrt(HD))

NCORES = 8
TP = 4                 # head-group shards
HL = H // TP           # 8 local q heads
KVL = KVH // TP        # 2 local kv heads
LHL = LH // TP         # 8 local latent heads

f32 = mybir.dt.float32
bf16 = mybir.dt.bfloat16

D_T = D // 128         # 32 k-tiles over model dim
S_T = S // 128         # 8 token tiles of 128
IB = 2                 # token blocks of 512
XT_CHUNK = 4           # kt per xt DMA chunk (8 chunks)


def _build_program():
    nc = bacc.Bacc("TRN2", target_bir_lowering=False, debug=False)

    xt_d = nc.dram_tensor("xt", [128, D_T, S], bf16, kind="ExternalInput")
    wq_d = nc.dram_tensor("wq", [HL, 128, D_T, 128], bf16, kind="ExternalInput")
    wk_d = nc.dram_tensor("wk", [KVL, 128, D_T, 128], bf16, kind="ExternalInput")
    wv_d = nc.dram_tensor("wv", [128, D_T, KVL * HD], bf16, kind="ExternalInput")
    wlk_d = nc.dram_tensor("wlk", [LHL, 128, D_T, 128], bf16, kind="ExternalInput")
    wlv_d = nc.dram_tensor("wlv", [128, D_T, LHL * HD], bf16, kind="ExternalInput")
    wo_d = nc.dram_tensor("wo", [8, 128, HL, 512], bf16, kind="ExternalInput")
    wlo_d = nc.dram_tensor("wlo", [8, 128, LHL, 512], bf16, kind="ExternalInput")
    cos_d = nc.dram_tensor("cosT", [HD, S], bf16, kind="ExternalInput")
    sin_d = nc.dram_tensor("sinTs", [HD, S], bf16, kind="ExternalInput")
    mask_d = nc.dram_tensor("maskd", [128, 128], bf16, kind="ExternalInput")
    out_d = nc.dram_tensor("out", [S, D], f32, kind="ExternalOutput")
    warm_d = nc.dram_tensor("warm", [128, 512], f32, kind="ExternalOutput")

    out_ap = out_d.ap().rearrange("(tt p) d -> p tt d", p=128)

    with tile.TileContext(nc) as tc:
        with tc.tile_pool(name="const", bufs=1) as constp, \
             tc.tile_pool(name="acts", bufs=1) as acts:

            ones = constp.tile([128, 512], bf16, tag="ones")
            nc.vector.memset(ones[:], 1.0)
            cosT = constp.tile([HD, S], bf16, tag="cosT")
            sinTs = constp.tile([HD, S], bf16, tag="sinTs")
            maskd = constp.tile([128, 128], bf16, tag="maskd")
            nc.sync.dma_start(cosT[:], cos_d.ap())
            nc.sync.dma_start(sinTs[:], sin_d.ap())
            nc.sync.dma_start(maskd[:], mask_d.ap())

            # persistent activations (bf16)
            kT = acts.tile([128, KVL, S], bf16, tag="kT")
            lkT = acts.tile([128, LHL, S], bf16, tag="lkT")
            v_sb = acts.tile([128, S_T, KVL * HD], bf16, tag="v")
            lv_sb = acts.tile([128, S_T, LHL * HD], bf16, tag="lv")
            attnT = acts.tile([128, HL, S], bf16, tag="attnT")
            latT = acts.tile([128, LHL, S], bf16, tag="latT")

            # ---- phase W: HAM warmup during initial DMA wait ----
            with tc.tile_pool(name="warm", bufs=1) as warmp, \
                 tc.tile_pool(name="ps_w", bufs=1, space="PSUM") as psw:
                ps_wt = psw.tile([128, 512], f32, tag="ps_w")
                NWARM = 20
                for i in range(NWARM):
                    nc.tensor.matmul(ps_wt[:], ones[:, 0:128], ones[:],
                                     start=(i == 0), stop=(i == NWARM - 1))
                wsb = warmp.tile([128, 512], f32, tag="wsb")
                nc.vector.tensor_copy(wsb[:], ps_wt[:])
                nc.gpsimd.dma_start(warm_d.ap(), wsb[:])

            with tc.tile_pool(name="xt", bufs=1) as xtp, \
                 tc.tile_pool(name="wstr", bufs=2) as wstr:
                xt = xtp.tile([128, D_T, S], bf16, tag="xt")

                # ---- phase B2: token-major projections v, lv ----
                # kt-outer passes (compute paces with the chunked DMAs),
                # capped at 6 PSUM banks so the next phase can start while
                # the last pass drains.
                with tc.tile_pool(name="wvlv", bufs=1) as wvp, \
                     tc.tile_pool(name="ps_b2", bufs=6, space="PSUM") as psb2:
                    # allocation order matters: the stack allocator reclaims
                    # top-down, so wv (whose readers finish 40us before the
                    # wlv half-1 readers) must sit on top for the next
                    # phase's weight-stream pool to reuse its range early.
                    wlv_sb = [wvp.tile([128, D_T, 512], bf16, tag="wlvh",
                                       name=f"wlvh_{h}") for h in range(2)]
                    wv_sb = wvp.tile([128, D_T, KVL * HD], bf16, tag="wv")
                    # interleaved startup DMAs: xt / wv / wlv half0 by chunk.
                    # Only the first xt chunk is split per-kt: one queue only
                    # sustains ~1/16 of HBM BW, so splitting gets the first
                    # matmul's operands in flight on parallel queues, while
                    # keeping the total dma_start count low (sync-engine
                    # issue is ~0.6us per dma_start).
                    for kt in range(XT_CHUNK):
                        ks = bass.ds(kt, 1)
                        nc.sync.dma_start(xt[:, ks, :], xt_d.ap()[:, ks, :])
                    nc.sync.dma_start(wv_sb[:, 0:XT_CHUNK, :],
                                      wv_d.ap()[:, 0:XT_CHUNK, :])
                    for kt in range(0, XT_CHUNK, 2):
                        nc.sync.dma_start(wlv_sb[0][:, kt:kt + 2, :],
                                          wlv_d.ap()[:, kt:kt + 2, 0:512])
                    for c in range(1, D_T // XT_CHUNK):
                        cs = bass.ds(c * XT_CHUNK, XT_CHUNK)
                        nc.sync.dma_start(xt[:, cs, :], xt_d.ap()[:, cs, :])
                        nc.sync.dma_start(wv_sb[:, cs, :],
                                          wv_d.ap()[:, cs, :])
                        nc.sync.dma_start(wlv_sb[0][:, cs, :],
                                          wlv_d.ap()[:, cs, 0:512])

                    # half 0: lv + v together (shared stationary xt tiles)
                    for tts in ([0, 1, 2], [3, 4, 5], [6, 7]):
                        ps_lv = {tt: psb2.tile([128, 512], f32, tag="ps_b2",
                                               name=f"ps_lv0_{tt}")
                                 for tt in tts}
                        ps_v = {tt: psb2.tile([128, KVL * HD], f32,
                                              tag="ps_b2", name=f"ps_v_{tt}")
                                for tt in tts}
                        for kt in range(D_T):
                            st, sp = kt == 0, kt == D_T - 1
                            for tt in tts:
                                lhs = xt[:, kt, bass.ts(tt, 128)]
                                nc.tensor.matmul(ps_lv[tt][:], lhs,
                                                 wlv_sb[0][:, kt, :],
                                                 start=st, stop=sp)
                                nc.tensor.matmul(ps_v[tt][:], lhs,
                                                 wv_sb[:, kt, :],
                                                 start=st, stop=sp)
                        for tt in tts:
                            nc.vector.tensor_copy(lv_sb[:, tt, 0:512],
                                                  ps_lv[tt][:])
                            nc.vector.tensor_copy(v_sb[:, tt, :], ps_v[tt][:])

                    # half-1 DMA (chunked; slot reuses only after pass 2 reads)
                    for c in range(D_T // XT_CHUNK):
                        cs = bass.ds(c * XT_CHUNK, XT_CHUNK)
                        nc.sync.dma_start(wlv_sb[1][:, cs, :],
                                          wlv_d.ap()[:, cs, 512:1024])
                    # half 1 of lv
                    for tts in ([0, 1, 2, 3, 4, 5], [6, 7]):
                        ps_lv = {tt: psb2.tile([128, 512], f32, tag="ps_b2",
                                               name=f"ps_lv1_{tt}")
                                 for tt in tts}
                        for kt in range(D_T):
                            st, sp = kt == 0, kt == D_T - 1
                            for tt in tts:
                                nc.tensor.matmul(ps_lv[tt][:],
                                                 xt[:, kt, bass.ts(tt, 128)],
                                                 wlv_sb[1][:, kt, :],
                                                 start=st, stop=sp)
                        for tt in tts:
                            nc.vector.tensor_copy(lv_sb[:, tt, 512:1024],
                                                  ps_lv[tt][:])

                # ---- phases B1 + C (software-pipelined per head) ----
                with tc.tile_pool(name="qq", bufs=2) as qqp, \
                     tc.tile_pool(name="pp", bufs=14) as pp, \
                     tc.tile_pool(name="f32t", bufs=3) as f32t, \
                     tc.tile_pool(name="dn", bufs=2) as dn, \
                     tc.tile_pool(name="dnb", bufs=2) as dnb, \
                     tc.tile_pool(name="ps_s", bufs=3, space="PSUM") as pss_, \
                     tc.tile_pool(name="ps_o", bufs=2, space="PSUM") as pso_, \
                     tc.tile_pool(name="ps_d", bufs=1, space="PSUM") as psd_, \
                     tc.tile_pool(name="ps_b1", bufs=2, space="PSUM") as psb1:

                    def rope_to(dst, ps, ib):
                        sl = bass.ts(ib, 512)
                        rt = f32t.tile([128, 512], f32, tag="f32t")
                        qc = f32t.tile([128, 512], f32, tag="f32t")
                        nc.vector.tensor_tensor(
                            rt[0:64, :], ps[64:128, :], sinTs[0:64, sl],
                            mybir.AluOpType.mult)
                        nc.vector.tensor_tensor(
                            rt[64:128, :], ps[0:64, :], sinTs[64:128, sl],
                            mybir.AluOpType.mult)
                        nc.vector.tensor_tensor(
                            qc[:], ps[:], cosT[:, sl], mybir.AluOpType.mult)
                        nc.vector.tensor_add(dst, qc[:], rt[:])

                    def proj_dma(w_dram, nt):
                        wt = wstr.tile([128, D_T, 128], bf16, tag="w")
                        nc.sync.dma_start(wt[:], w_dram.ap()[nt])
                        return wt

                    def proj_half(wt, key, dst, ib):
                        # dst ib-half = rope(wt.T @ xt), one PSUM bank
                        ps = psb1.tile([128, 512], f32, tag="ps_b1",
                                       name=f"ps_b1_{key}_{ib}")
                        for kt in range(D_T):
                            nc.tensor.matmul(
                                ps[:], wt[:, kt, :],
                                xt[:, kt, bass.ts(ib, 512)],
                                start=(kt == 0), stop=(kt == D_T - 1))
                        rope_to(dst[:, bass.ts(ib, 512)], ps[:], ib)

                    def proj_head(w_dram, nt, dst, key):
                        wt = proj_dma(w_dram, nt)
                        for ib in range(IB):
                            proj_half(wt, f"{key}{nt}", dst, ib)

                    # attention unit = (head dst, ib block); three emission
                    # stages so exp latency hides under projection matmuls
                    def u_off(u, jb):
                        # diagonal j-blocks only need queries i >= 128*jb:
                        # use a shorter moving stream into a nested PSUM
                        # sub-range (later matmuls overwrite-or-accumulate
                        # per-element via the has_written bits).
                        r = jb - 4 * u["ib"]
                        return 128 * r if r >= 0 else 0

                    def u_scores(u):
                        qsrc, ksrc, ib = u["qsrc"], u["ksrc"], u["ib"]
                        for jb in range(u["njb"]):
                            off = u_off(u, jb)
                            sl = bass.ds(off, 512 - off)
                            ps_s = pss_.tile([128, 512], f32, tag="ps_s")
                            nc.tensor.matmul(
                                ps_s[:, sl], ksrc[:, bass.ts(jb, 128)],
                                qsrc[:, bass.ds(ib * 512 + off, 512 - off)],
                                start=True, stop=True)
                            pt = pp.tile([128, 512], bf16, tag="pt")
                            nc.scalar.activation(
                                pt[:, sl], ps_s[:, sl],
                                mybir.ActivationFunctionType.Exp,
                                scale=SCALE)
                            if jb - 4 * ib >= 0:
                                # causal mask on the first 128 cols (j > i)
                                nc.vector.tensor_tensor(
                                    pt[:, bass.ds(off, 128)],
                                    pt[:, bass.ds(off, 128)], maskd[:],
                                    mybir.AluOpType.mult)
                            u["pts"].append(pt)

                    def u_den(u):
                        # softmax denominator: ones-matmuls -> [1,512]
                        # -> fast reciprocal -> gpsimd broadcast
                        ps_d = psd_.tile([1, 512], f32, tag="ps_d")
                        for jb in range(u["njb"]):
                            off = u_off(u, jb)
                            sl = bass.ds(off, 512 - off)
                            nc.tensor.matmul(
                                ps_d[:, sl], ones[:, 0:1], u["pts"][jb][:, sl],
                                start=(jb == 0), stop=(jb == u["njb"] - 1))
                        rec = dn.tile([1, 512], f32, tag="rec")
                        nc.vector.reciprocal_approx_fast(rec[:], ps_d[:])
                        recb = dnb.tile([128, 512], f32, tag="recb")
                        nc.gpsimd.partition_broadcast(recb[:], rec[:])
                        u["recb"] = recb

                    def u_pv(u):
                        vsrc, vofs, ib = u["vsrc"], u["vofs"], u["ib"]
                        ps_o = pso_.tile([128, 512], f32, tag="ps_o")
                        for jb in range(u["njb"]):
                            off = u_off(u, jb)
                            sl = bass.ds(off, 512 - off)
                            nc.tensor.matmul(
                                ps_o[:, sl], vsrc[:, jb, vofs],
                                u["pts"][jb][:, sl],
                                start=(jb == 0), stop=(jb == u["njb"] - 1))
                        nc.vector.tensor_tensor(
                            u["dst"][:, bass.ts(ib, 512)], ps_o[:],
                            u["recb"][:], mybir.AluOpType.mult)

                    def mk_units(h, qsrc):
                        out = []
                        for latent in (False, True):
                            for ib in range(IB):
                                out.append({
                                    "qsrc": qsrc, "ib": ib,
                                    "njb": 4 * (ib + 1), "pts": [],
                                    "ksrc": (lkT[:, h, :] if latent
                                             else kT[:, h // GROUPS, :]),
                                    "vsrc": lv_sb if latent else v_sb,
                                    "vofs": bass.ts(h if latent
                                                    else h // GROUPS, HD),
                                    "dst": (latT[:, h, :] if latent
                                            else attnT[:, h, :]),
                                })
                        return out

                    # B1-k: kv heads (feature-major, roped)
                    for nt in range(KVL):
                        proj_head(wk_d, nt, kT[:, nt, :], "k")
                    # q head 0 early (its weight DMA slots are free here)
                    qq = qqp.tile([128, S], bf16, tag="qq")
                    proj_head(wq_d, 0, qq[:], "q")
                    # B1-lk: latent-key heads via composed w_lq@w_lk
                    for nt in range(LHL):
                        proj_head(wlk_d, nt, lkT[:, nt, :], "lk")

                    # pipelined heads: for head h emit scores/exp early,
                    # weave next head's projection between, PV after.
                    for h in range(HL):
                        A0, A1, L0, L1 = units = mk_units(h, qq)
                        if h + 1 < HL:
                            qq = qqp.tile([128, S], bf16, tag="qq")
                            wt_next = proj_dma(wq_d, h + 1)
                        u_scores(A0)
                        u_scores(A1)
                        u_scores(L0)
                        u_den(A0)
                        u_scores(L1)
                        u_den(A1)
                        if h + 1 < HL:
                            proj_half(wt_next, f"q{h+1}", qq[:], 0)
                        u_den(L0)
                        u_pv(A0)
                        u_pv(A1)
                        if h + 1 < HL:
                            proj_half(wt_next, f"q{h+1}", qq[:], 1)
                        u_den(L1)
                        u_pv(L0)
                        u_pv(L1)

            # ---- phase D: output projections (row-sharded, partial sum) ----
            with tc.tile_pool(name="wop", bufs=2) as wop, \
                 tc.tile_pool(name="ost", bufs=4) as ost, \
                 tc.tile_pool(name="ps_f", bufs=4, space="PSUM") as psf:
                for np_ in range(4):       # pairs of 512-wide col blocks
                    wo2 = wop.tile([128, HL, 1024], bf16, tag="wo2")
                    wlo2 = wop.tile([128, LHL, 1024], bf16, tag="wlo2")
                    for u in range(2):
                        for h in range(HL):     # per-head chunks: D's first
                            nc.sync.dma_start(  # MMs start after ~0.7us
                                wo2[:, h, bass.ts(u, 512)],
                                wo_d.ap()[2 * np_ + u][:, h, :])
                        for h in range(LHL):
                            nc.sync.dma_start(
                                wlo2[:, h, bass.ts(u, 512)],
                                wlo_d.ap()[2 * np_ + u][:, h, :])
                    for tt in range(S_T):
                        ps0 = psf.tile([128, 512], f32, tag="ps_f")
                        ps1 = psf.tile([128, 512], f32, tag="ps_f")
                        for h in range(HL):
                            lhs = attnT[:, h, bass.ts(tt, 128)]
                            nc.tensor.matmul(ps0[:], lhs,
                                             wo2[:, h, 0:512],
                                             start=(h == 0), stop=False)
                            nc.tensor.matmul(ps1[:], lhs,
                                             wo2[:, h, 512:1024],
                                             start=(h == 0), stop=False)
                        for h in range(LHL):
                            lhs = latT[:, h, bass.ts(tt, 128)]
                            nc.tensor.matmul(ps0[:], lhs,
                                             wlo2[:, h, 0:512],
                                             start=False, stop=(h == LHL - 1))
                            nc.tensor.matmul(ps1[:], lhs,
                                             wlo2[:, h, 512:1024],
                                             start=False, stop=(h == LHL - 1))
                        for u, ps in enumerate((ps0, ps1)):
                            ot = ost.tile([128, 512], f32, tag="ot")
                            nc.vector.tensor_copy(ot[:], ps[:])
                            nc.sync.dma_start(
                                out_ap[:, tt, bass.ds(
                                    (2 * np_ + u) * 512, 512)],
                                ot[:])

    nc.compile()
    return nc


_NC = None


def _get_program():
    global _NC
    if _NC is None:
        _NC = _build_program()
    return _NC


def _rope_tables():
    inv_freq = 1.0 / (THETA ** (np.arange(0, HD, 2, dtype=np.float32) / HD))
    t = np.arange(S, dtype=np.float32)
    freqs = np.outer(t, inv_freq)                       # [S, 64]
    emb = np.concatenate([freqs, freqs], axis=-1)       # [S, HD]
    cosT = np.cos(emb).T.astype(BF16).copy()            # [HD, S]
    sinT = np.sin(emb).T.astype(np.float32)
    sinTs = np.concatenate([-sinT[:HD // 2], sinT[HD // 2:]], 0).astype(
        BF16).copy()
    return cosT, sinTs


def _mask_patterns():
    # maskd[p, i] = 1.0 iff p <= i (diagonal 128x128 causal block)
    p = np.arange(128)[:, None]
    i = np.arange(128)[None, :]
    return (p <= i).astype(BF16)


def _tile_w_fm(w, n_tiles, kt):
    # [K, n_tiles*128] -> [n_tiles, 128(p of K), kt, 128]
    K, N = w.shape
    assert K == kt * 128 and N == n_tiles * 128
    return np.ascontiguousarray(
        w.reshape(kt, 128, n_tiles, 128).transpose(2, 1, 0, 3)).astype(BF16)


def _tile_w_tm(w, kt):
    # [K, N] -> [128(p of K), kt, N]
    K, N = w.shape
    assert K == kt * 128
    return np.ascontiguousarray(
        w.reshape(kt, 128, N).transpose(1, 0, 2)).astype(BF16)


def _tile_w_out(w):
    # [1024, D] -> [8(nb), 128(p of rows), 8(h), 512]
    return np.ascontiguousarray(
        w.reshape(8, 128, D // 512, 512).transpose(2, 1, 0, 3)).astype(BF16)


def build_in_maps(hidden_states, w_q, w_k, w_v, w_o, w_lq, w_lk, w_lv, w_lo):
    cosT, sinTs = _rope_tables()
    maskd = _mask_patterns()
    # compose the latent-q/latent-k projections: lk = x @ (w_lq @ w_lk)
    wlqk = np.asarray(w_lq, np.float32) @ np.asarray(w_lk, np.float32)

    in_maps = []
    for c in range(NCORES):
        b, g = divmod(c, TP)
        x = np.asarray(hidden_states[b], dtype=np.float32)       # [S, D]
        xt = np.ascontiguousarray(
            x.T.reshape(D_T, 128, S).transpose(1, 0, 2)).astype(BF16)
        qs = slice(g * HL * HD, (g + 1) * HL * HD)
        kvs = slice(g * KVL * HD, (g + 1) * KVL * HD)
        ls = slice(g * LHL * HD, (g + 1) * LHL * HD)
        in_maps.append({
            "xt": xt,
            "wq": _tile_w_fm(np.asarray(w_q)[:, qs], HL, D_T),
            "wk": _tile_w_fm(np.asarray(w_k)[:, kvs], KVL, D_T),
            "wv": _tile_w_tm(np.asarray(w_v)[:, kvs], D_T),
            "wlk": _tile_w_fm(wlqk[:, ls], LHL, D_T),
            "wlv": _tile_w_tm(np.asarray(w_lv)[:, ls], D_T),
            "wo": _tile_w_out(np.asarray(w_o)[qs, :]),
            "wlo": _tile_w_out(np.asarray(w_lo)[ls, :]),
            "cosT": cosT,
            "sinTs": sinTs,
            "maskd": maskd,
        })
    return in_maps


def kernel(hidden_states, w_q, w_k, w_v, w_o, w_lq, w_lk, w_lv, w_lo):
    nc = _get_program()
    in_maps = build_in_maps(hidden_states, w_q, w_k, w_v, w_o,
                            w_lq, w_lk, w_lv, w_lo)
    res = run_bass_kernel_spmd(nc, in_maps, list(range(NCORES))).results

    out = np.zeros((B, S, D), dtype=np.float32)
    for c in range(NCORES):
        b = c // TP
        out[b] += res[c]["out"]
    return out


# revision 19
# speedup vs baseline: 1.0047x; 1.0047x over previous
"""Trainium2 Bass kernel for LlamaMultiheadLatentAttention.

Contract: kernel(**inputs) takes FULL fp32 inputs (as produced by
reference.setup_inputs) and returns the FULL fp32 output [2, 1024, 4096].

Sharding (8 cores, no collectives): core c handles batch b = c//4 and
head-group g = c%4 (8 query heads, 2 kv heads, 8 latent heads). q/k/v and
latent projections are column-sharded per head-group; o_proj/latent_o_proj
are row-sharded, so each core emits a partial output sum and the host adds
the 4 partials per batch (the output all-reduce happens at unshard time).

Key design points (baseline 1.01ms -> ~0.75ms):
  - all matmul operands bf16 (fp8 was measured numerically infeasible for
    the 2e-2 gate: even w_v/w_lv alone in e4m3 gives rel=2.8e-2).
  - w_lq @ w_lk composed on the host into one weight: the lq GEMM
    disappears (-27us TensorE/core) and numerics slightly improve.
  - feature-major activations; attention scores computed transposed
    S^T[j,i] so softmax/PV need no transposes.
  - causal trimming by shortened moving streams: a diagonal j-block's
    score/PV/denominator matmuls only stream queries i >= 128*jb into a
    nested PSUM sub-range (the per-element has_written bits make
    overwrite-then-accumulate across nested ranges correct). -31us PE,
    -25% exp elements, masks shrink to one [128,128] multiply per block.
  - softmax denominator via ones-stationary matmuls -> [1,512]
    reciprocal_approx_fast -> gpsimd partition_broadcast (off the
    DVE/ACT critical path; v1's [1,512] InstReciprocal cost 3.3us each).
  - software-pipelined per-head emission: scores+exp early, next head's
    q-projection between, PV after, so ScalarE's ~110us of exp hides
    entirely under projection matmuls and HAM stays at K=8/8 for the
    whole kernel body.
  - pool/DMA orchestration: weight-stream pool allocated before the B2
    weight pool (new-pool writes wait on the WHOLE previous pool's
    release, which otherwise serializes the first B1 weight DMA behind
    all of B2); warmup-result DMA on the gpsimd queue (on the sync queue
    it head-of-line-blocks every input DMA); startup DMAs chunked and
    first chunks split across queues (one queue sustains ~1/16 of HBM
    BW); HAM warmup matmuls cover the initial DMA latency.
  - all PSUM->SBUF copies pinned to DVE (nc.any copies land on ScalarE
    which is both slow for copies and busy with exp).
"""

import numpy as np
import ml_dtypes

import concourse.bass as bass
import concourse.mybir as mybir
import concourse.tile as tile
from concourse import bacc
from concourse.bass_utils import run_bass_kernel_spmd

BF16 = ml_dtypes.bfloat16

B, S, D = 2, 1024, 4096
H, KVH, HD = 32, 8, 128
GROUPS = H // KVH
LAT, LH = 1024, 32
THETA = 10000.0
SCALE = 1.0 / float(np.sqrt(HD))

NCORES = 8
TP = 4                 # head-group shards
HL = H // TP           # 8 local q heads
KVL = KVH // TP        # 2 local kv heads
LHL = LH // TP         # 8 local latent heads

f32 = mybir.dt.float32
bf16 = mybir.dt.bfloat16

D_T = D // 128         # 32 k-tiles over model dim
S_T = S // 128         # 8 token tiles of 128
IB = 2                 # token blocks of 512
XT_CHUNK = 4           # kt per xt DMA chunk (8 chunks)


def _build_program():
    nc = bacc.Bacc("TRN2", target_bir_lowering=False, debug=False)

    xt_d = nc.dram_tensor("xt", [128, D_T, S], bf16, kind="ExternalInput")
    wq_d = nc.dram_tensor("wq", [HL, 128, D_T, 128], bf16, kind="ExternalInput")
    wk_d = nc.dram_tensor("wk", [KVL, 128, D_T, 128], bf16, kind="ExternalInput")
    wv_d = nc.dram_tensor("wv", [128, D_T, KVL * HD], bf16, kind="ExternalInput")
    wlk_d = nc.dram_tensor("wlk", [LHL, 128, D_T, 128], bf16, kind="ExternalInput")
    wlv_d = nc.dram_tensor("wlv", [128, D_T, LHL * HD], bf16, kind="ExternalInput")
    wo_d = nc.dram_tensor("wo", [8, 128, HL, 512], bf16, kind="ExternalInput")
    wlo_d = nc.dram_tensor("wlo", [8, 128, LHL, 512], bf16, kind="ExternalInput")
    cos_d = nc.dram_tensor("cosT", [HD, S], bf16, kind="ExternalInput")
    sin_d = nc.dram_tensor("sinTs", [HD, S], bf16, kind="ExternalInput")
    mask_d = nc.dram_tensor("maskd", [128, 128], bf16, kind="ExternalInput")
    out_d = nc.dram_tensor("out", [S, D], f32, kind="ExternalOutput")
    warm_d = nc.dram_tensor("warm", [128, 512], f32, kind="ExternalOutput")

    out_ap = out_d.ap().rearrange("(tt p) d -> p tt d", p=128)

    with tile.TileContext(nc) as tc:
        with tc.tile_pool(name="const", bufs=1) as constp, \
             tc.tile_pool(name="acts", bufs=1) as acts:

            ones = constp.tile([128, 512], bf16, tag="ones")
            nc.vector.memset(ones[:], 1.0)
            cosT = constp.tile([HD, S], bf16, tag="cosT")
            sinTs = constp.tile([HD, S], bf16, tag="sinTs")
            maskd = constp.tile([128, 128], bf16, tag="maskd")
            nc.sync.dma_start(cosT[:], cos_d.ap())
            nc.sync.dma_start(sinTs[:], sin_d.ap())
            nc.sync.dma_start(maskd[:], mask_d.ap())

            # persistent activations (bf16)
            kT = acts.tile([128, KVL, S], bf16, tag="kT")
            lkT = acts.tile([128, LHL, S], bf16, tag="lkT")
            v_sb = acts.tile([128, S_T, KVL * HD], bf16, tag="v")
            lv_sb = acts.tile([128, S_T, LHL * HD], bf16, tag="lv")
            attnT = acts.tile([128, HL, S], bf16, tag="attnT")
            latT = acts.tile([128, LHL, S], bf16, tag="latT")

            # ---- phase W: HAM warmup during initial DMA wait ----
            with tc.tile_pool(name="warm", bufs=1) as warmp, \
                 tc.tile_pool(name="ps_w", bufs=1, space="PSUM") as psw:
                ps_wt = psw.tile([128, 512], f32, tag="ps_w")
                NWARM = 12
                for i in range(NWARM):
                    nc.tensor.matmul(ps_wt[:], ones[:, 0:128], ones[:],
                                     start=(i == 0), stop=(i == NWARM - 1))
                wsb = warmp.tile([128, 512], f32, tag="wsb")
                nc.vector.tensor_copy(wsb[:], ps_wt[:])
                nc.gpsimd.dma_start(warm_d.ap(), wsb[:])

            with tc.tile_pool(name="xt", bufs=1) as xtp, \
                 tc.tile_pool(name="wstr", bufs=2) as wstr:
                xt = xtp.tile([128, D_T, S], bf16, tag="xt")

                # ---- phase B2: token-major projections v, lv ----
                # kt-outer passes (compute paces with the chunked DMAs),
                # capped at 6 PSUM banks so the next phase can start while
                # the last pass drains.
                with tc.tile_pool(name="wvlv", bufs=1) as wvp, \
                     tc.tile_pool(name="ps_b2", bufs=6, space="PSUM") as psb2:
                    # allocation order matters: the stack allocator reclaims
                    # top-down, so wv (whose readers finish 40us before the
                    # wlv half-1 readers) must sit on top for the next
                    # phase's weight-stream pool to reuse its range early.
                    wlv_sb = [wvp.tile([128, D_T, 512], bf16, tag="wlvh",
                                       name=f"wlvh_{h}") for h in range(2)]
                    wv_sb = wvp.tile([128, D_T, KVL * HD], bf16, tag="wv")
                    # interleaved startup DMAs: xt / wv / wlv half0 by chunk.
                    # Only the first xt chunk is split per-kt: one queue only
                    # sustains ~1/16 of HBM BW, so splitting gets the first
                    # matmul's operands in flight on parallel queues, while
                    # keeping the total dma_start count low (sync-engine
                    # issue is ~0.6us per dma_start).
                    nc.sync.dma_start(xt[:, 0:1, 0:512],
                                      xt_d.ap()[:, 0:1, 0:512])
                    nc.sync.dma_start(wv_sb[:, 0:1, :], wv_d.ap()[:, 0:1, :])
                    nc.sync.dma_start(wlv_sb[0][:, 0:1, :],
                                      wlv_d.ap()[:, 0:1, 0:512])
                    nc.sync.dma_start(xt[:, 0:1, 512:1024],
                                      xt_d.ap()[:, 0:1, 512:1024])
                    for kt in range(1, XT_CHUNK):
                        ks = bass.ds(kt, 1)
                        nc.sync.dma_start(xt[:, ks, :], xt_d.ap()[:, ks, :])
                        nc.sync.dma_start(wv_sb[:, ks, :], wv_d.ap()[:, ks, :])
                        nc.sync.dma_start(wlv_sb[0][:, ks, :],
                                          wlv_d.ap()[:, ks, 0:512])
                    for c in range(1, D_T // XT_CHUNK):
                        cs = bass.ds(c * XT_CHUNK, XT_CHUNK)
                        nc.sync.dma_start(xt[:, cs, :], xt_d.ap()[:, cs, :])
                        nc.sync.dma_start(wv_sb[:, cs, :],
                                          wv_d.ap()[:, cs, :])
                        nc.sync.dma_start(wlv_sb[0][:, cs, :],
                                          wlv_d.ap()[:, cs, 0:512])

                    # half 0: lv + v together (shared stationary xt tiles)
                    for tts in ([0, 1, 2], [3, 4, 5], [6, 7]):
                        ps_lv = {tt: psb2.tile([128, 512], f32, tag="ps_b2",
                                               name=f"ps_lv0_{tt}")
                                 for tt in tts}
                        ps_v = {tt: psb2.tile([128, KVL * HD], f32,
                                              tag="ps_b2", name=f"ps_v_{tt}")
                                for tt in tts}
                        for kt in range(D_T):
                            st, sp = kt == 0, kt == D_T - 1
                            for tt in tts:
                                lhs = xt[:, kt, bass.ts(tt, 128)]
                                nc.tensor.matmul(ps_lv[tt][:], lhs,
                                                 wlv_sb[0][:, kt, :],
                                                 start=st, stop=sp)
                                nc.tensor.matmul(ps_v[tt][:], lhs,
                                                 wv_sb[:, kt, :],
                                                 start=st, stop=sp)
                        for tt in tts:
                            nc.vector.tensor_copy(lv_sb[:, tt, 0:512],
                                                  ps_lv[tt][:])
                            nc.vector.tensor_copy(v_sb[:, tt, :], ps_v[tt][:])

                    # half-1 DMA (chunked; slot reuses only after pass 2 reads)
                    for c in range(D_T // XT_CHUNK):
                        cs = bass.ds(c * XT_CHUNK, XT_CHUNK)
                        nc.sync.dma_start(wlv_sb[1][:, cs, :],
                                          wlv_d.ap()[:, cs, 512:1024])
                    # half 1 of lv
                    for tts in ([0, 1, 2, 3, 4, 5], [6, 7]):
                        ps_lv = {tt: psb2.tile([128, 512], f32, tag="ps_b2",
                                               name=f"ps_lv1_{tt}")
                                 for tt in tts}
                        for kt in range(D_T):
                            st, sp = kt == 0, kt == D_T - 1
                            for tt in tts:
                                nc.tensor.matmul(ps_lv[tt][:],
                                                 xt[:, kt, bass.ts(tt, 128)],
                                                 wlv_sb[1][:, kt, :],
                                                 start=st, stop=sp)
                        for tt in tts:
                            nc.vector.tensor_copy(lv_sb[:, tt, 512:1024],
                                                  ps_lv[tt][:])

                # ---- phases B1 + C (software-pipelined per head) ----
                with tc.tile_pool(name="qq", bufs=2) as qqp, \
                     tc.tile_pool(name="pp", bufs=14) as pp, \
                     tc.tile_pool(name="f32t", bufs=3) as f32t, \
                     tc.tile_pool(name="dn", bufs=2) as dn, \
                     tc.tile_pool(name="dnb", bufs=2) as dnb, \
                     tc.tile_pool(name="ps_s", bufs=3, space="PSUM") as pss_, \
                     tc.tile_pool(name="ps_o", bufs=2, space="PSUM") as pso_, \
                     tc.tile_pool(name="ps_d", bufs=1, space="PSUM") as psd_, \
                     tc.tile_pool(name="ps_b1", bufs=2, space="PSUM") as psb1:

                    def rope_to(dst, ps, ib):
                        sl = bass.ts(ib, 512)
                        rt = f32t.tile([128, 512], f32, tag="f32t")
                        qc = f32t.tile([128, 512], f32, tag="f32t")
                        nc.vector.tensor_tensor(
                            rt[0:64, :], ps[64:128, :], sinTs[0:64, sl],
                            mybir.AluOpType.mult)
                        nc.vector.tensor_tensor(
                            rt[64:128, :], ps[0:64, :], sinTs[64:128, sl],
                            mybir.AluOpType.mult)
                        nc.vector.tensor_tensor(
                            qc[:], ps[:], cosT[:, sl], mybir.AluOpType.mult)
                        nc.vector.tensor_add(dst, qc[:], rt[:])

                    def proj_dma(w_dram, nt):
                        wt = wstr.tile([128, D_T, 128], bf16, tag="w")
                        nc.sync.dma_start(wt[:], w_dram.ap()[nt])
                        return wt

                    def proj_half(wt, key, dst, ib):
                        # dst ib-half = rope(wt.T @ xt), one PSUM bank
                        ps = psb1.tile([128, 512], f32, tag="ps_b1",
                                       name=f"ps_b1_{key}_{ib}")
                        for kt in range(D_T):
                            nc.tensor.matmul(
                                ps[:], wt[:, kt, :],
                                xt[:, kt, bass.ts(ib, 512)],
                                start=(kt == 0), stop=(kt == D_T - 1))
                        rope_to(dst[:, bass.ts(ib, 512)], ps[:], ib)

                    def proj_head(w_dram, nt, dst, key):
                        wt = proj_dma(w_dram, nt)
                        for ib in range(IB):
                            proj_half(wt, f"{key}{nt}", dst, ib)

                    # attention unit = (head dst, ib block); three emission
                    # stages so exp latency hides under projection matmuls
                    def u_off(u, jb):
                        # diagonal j-blocks only need queries i >= 128*jb:
                        # use a shorter moving stream into a nested PSUM
                        # sub-range (later matmuls overwrite-or-accumulate
                        # per-element via the has_written bits).
                        r = jb - 4 * u["ib"]
                        return 128 * r if r >= 0 else 0

                    def u_scores(u):
                        qsrc, ksrc, ib = u["qsrc"], u["ksrc"], u["ib"]
                        for jb in range(u["njb"]):
                            off = u_off(u, jb)
                            sl = bass.ds(off, 512 - off)
                            ps_s = pss_.tile([128, 512], f32, tag="ps_s")
                            nc.tensor.matmul(
                                ps_s[:, sl], ksrc[:, bass.ts(jb, 128)],
                                qsrc[:, bass.ds(ib * 512 + off, 512 - off)],
                                start=True, stop=True)
                            pt = pp.tile([128, 512], bf16, tag="pt")
                            nc.scalar.activation(
                                pt[:, sl], ps_s[:, sl],
                                mybir.ActivationFunctionType.Exp,
                                scale=SCALE)
                            if jb - 4 * ib >= 0:
                                # causal mask on the first 128 cols (j > i)
                                nc.vector.tensor_tensor(
                                    pt[:, bass.ds(off, 128)],
                                    pt[:, bass.ds(off, 128)], maskd[:],
                                    mybir.AluOpType.mult)
                            u["pts"].append(pt)

                    def u_den(u):
                        # softmax denominator: ones-matmuls -> [1,512]
                        # -> fast reciprocal -> gpsimd broadcast
                        ps_d = psd_.tile([1, 512], f32, tag="ps_d")
                        for jb in range(u["njb"]):
                            off = u_off(u, jb)
                            sl = bass.ds(off, 512 - off)
                            nc.tensor.matmul(
                                ps_d[:, sl], ones[:, 0:1], u["pts"][jb][:, sl],
                                start=(jb == 0), stop=(jb == u["njb"] - 1))
                        rec = dn.tile([1, 512], f32, tag="rec")
                        nc.vector.reciprocal_approx_fast(rec[:], ps_d[:])
                        recb = dnb.tile([128, 512], f32, tag="recb")
                        nc.gpsimd.partition_broadcast(recb[:], rec[:])
                        u["recb"] = recb

                    def u_pv(u):
                        vsrc, vofs, ib = u["vsrc"], u["vofs"], u["ib"]
                        ps_o = pso_.tile([128, 512], f32, tag="ps_o")
                        for jb in range(u["njb"]):
                            off = u_off(u, jb)
                            sl = bass.ds(off, 512 - off)
                            nc.tensor.matmul(
                                ps_o[:, sl], vsrc[:, jb, vofs],
                                u["pts"][jb][:, sl],
                                start=(jb == 0), stop=(jb == u["njb"] - 1))
                        nc.vector.tensor_tensor(
                            u["dst"][:, bass.ts(ib, 512)], ps_o[:],
                            u["recb"][:], mybir.AluOpType.mult)

                    def mk_units(h, qsrc):
                        out = []
                        for latent in (False, True):
                            for ib in range(IB):
                                out.append({
                                    "qsrc": qsrc, "ib": ib,
                                    "njb": 4 * (ib + 1), "pts": [],
                                    "ksrc": (lkT[:, h, :] if latent
                                             else kT[:, h // GROUPS, :]),
                                    "vsrc": lv_sb if latent else v_sb,
                                    "vofs": bass.ts(h if latent
                                                    else h // GROUPS, HD),
                                    "dst": (latT[:, h, :] if latent
                                            else attnT[:, h, :]),
                                })
                        return out

                    # B1-k: kv heads (feature-major, roped)
                    for nt in range(KVL):
                        proj_head(wk_d, nt, kT[:, nt, :], "k")
                    # q head 0 early (its weight DMA slots are free here)
                    qq = qqp.tile([128, S], bf16, tag="qq")
                    proj_head(wq_d, 0, qq[:], "q")
                    # B1-lk: latent-key heads via composed w_lq@w_lk
                    for nt in range(LHL):
                        proj_head(wlk_d, nt, lkT[:, nt, :], "lk")

                    # pipelined heads: for head h emit scores/exp early,
                    # weave next head's projection between, PV after.
                    for h in range(HL):
                        A0, A1, L0, L1 = units = mk_units(h, qq)
                        if h + 1 < HL:
                            qq = qqp.tile([128, S], bf16, tag="qq")
                            wt_next = proj_dma(wq_d, h + 1)
                        u_scores(A0)
                        u_scores(A1)
                        u_scores(L0)
                        u_den(A0)
                        u_scores(L1)
                        u_den(A1)
                        if h + 1 < HL:
                            proj_half(wt_next, f"q{h+1}", qq[:], 0)
                        u_den(L0)
                        u_pv(A0)
                        u_pv(A1)
                        if h + 1 < HL:
                            proj_half(wt_next, f"q{h+1}", qq[:], 1)
                        u_den(L1)
                        u_pv(L0)
                        u_pv(L1)

            # ---- phase D: output projections (row-sharded, partial sum) ----
            with tc.tile_pool(name="wop", bufs=2) as wop, \
                 tc.tile_pool(name="ost", bufs=4) as ost, \
                 tc.tile_pool(name="ps_f", bufs=4, space="PSUM") as psf:
                for np_ in range(4):       # pairs of 512-wide col blocks
                    wo2 = wop.tile([128, HL, 1024], bf16, tag="wo2")
                    wlo2 = wop.tile([128, LHL, 1024], bf16, tag="wlo2")
                    for u in range(2):
                        for h in range(HL):     # per-head chunks: D's first
                            nc.sync.dma_start(  # MMs start after ~0.7us
                                wo2[:, h, bass.ts(u, 512)],
                                wo_d.ap()[2 * np_ + u][:, h, :])
                        for h in range(LHL):
                            nc.sync.dma_start(
                                wlo2[:, h, bass.ts(u, 512)],
                                wlo_d.ap()[2 * np_ + u][:, h, :])
                    for tt in range(S_T):
                        ps0 = psf.tile([128, 512], f32, tag="ps_f")
                        ps1 = psf.tile([128, 512], f32, tag="ps_f")
                        for h in range(HL):
                            lhs = attnT[:, h, bass.ts(tt, 128)]
                            nc.tensor.matmul(ps0[:], lhs,
                                             wo2[:, h, 0:512],
                                             start=(h == 0), stop=False)
                            nc.tensor.matmul(ps1[:], lhs,
                                             wo2[:, h, 512:1024],
                                             start=(h == 0), stop=False)
                        for h in range(LHL):
                            lhs = latT[:, h, bass.ts(tt, 128)]
                            nc.tensor.matmul(ps0[:], lhs,
                                             wlo2[:, h, 0:512],
                                             start=False, stop=(h == LHL - 1))
                            nc.tensor.matmul(ps1[:], lhs,
                                             wlo2[:, h, 512:1024],
                                             start=False, stop=(h == LHL - 1))
                        for u, ps in enumerate((ps0, ps1)):
                            ot = ost.tile([128, 512], f32, tag="ot")
                            nc.vector.tensor_copy(ot[:], ps[:])
                            nc.sync.dma_start(
                                out_ap[:, tt, bass.ds(
                                    (2 * np_ + u) * 512, 512)],
                                ot[:])

    nc.compile()
    return nc


_NC = None


def _get_program():
    global _NC
    if _NC is None:
        _NC = _build_program()
    return _NC


def _rope_tables():
    inv_freq = 1.0 / (THETA ** (np.arange(0, HD, 2, dtype=np.float32) / HD))
    t = np.arange(S, dtype=np.float32)
    freqs = np.outer(t, inv_freq)                       # [S, 64]
    emb = np.concatenate([freqs, freqs], axis=-1)       # [S, HD]
    cosT = np.cos(emb).T.astype(BF16).copy()            # [HD, S]
    sinT = np.sin(emb).T.astype(np.float32)
    sinTs = np.concatenate([-sinT[:HD // 2], sinT[HD // 2:]], 0).astype(
        BF16).copy()
    return cosT, sinTs


def _mask_patterns():
    # maskd[p, i] = 1.0 iff p <= i (diagonal 128x128 causal block)
    p = np.arange(128)[:, None]
    i = np.arange(128)[None, :]
    return (p <= i).astype(BF16)


def _tile_w_fm(w, n_tiles, kt):
    # [K, n_tiles*128] -> [n_tiles, 128(p of K), kt, 128]
    K, N = w.shape
    assert K == kt * 128 and N == n_tiles * 128
    return np.ascontiguousarray(
        w.reshape(kt, 128, n_tiles, 128).transpose(2, 1, 0, 3)).astype(BF16)


def _tile_w_tm(w, kt):
    # [K, N] -> [128(p of K), kt, N]
    K, N = w.shape
    assert K == kt * 128
    return np.ascontiguousarray(
        w.reshape(kt, 128, N).transpose(1, 0, 2)).astype(BF16)


def _tile_w_out(w):
    # [1024, D] -> [8(nb), 128(p of rows), 8(h), 512]
    return np.ascontiguousarray(
        w.reshape(8, 128, D // 512, 512).transpose(2, 1, 0, 3)).astype(BF16)


def build_in_maps(hidden_states, w_q, w_k, w_v, w_o, w_lq, w_lk, w_lv, w_lo):
    cosT, sinTs = _rope_tables()
    maskd = _mask_patterns()
    # compose the latent-q/latent-k projections: lk = x @ (w_lq @ w_lk)
    wlqk = np.asarray(w_lq, np.float32) @ np.asarray(w_lk, np.float32)

    in_maps = []
    for c in range(NCORES):
        b, g = divmod(c, TP)
        x = np.asarray(hidden_states[b], dtype=np.float32)       # [S, D]
        xt = np.ascontiguousarray(
            x.T.reshape(D_T, 128, S).transpose(1, 0, 2)).astype(BF16)
        qs = slice(g * HL * HD, (g + 1) * HL * HD)
        kvs = slice(g * KVL * HD, (g + 1) * KVL * HD)
        ls = slice(g * LHL * HD, (g + 1) * LHL * HD)
        in_maps.append({
            "xt": xt,
            "wq": _tile_w_fm(np.asarray(w_q)[:, qs], HL, D_T),
            "wk": _tile_w_fm(np.asarray(w_k)[:, kvs], KVL, D_T),
            "wv": _tile_w_tm(np.asarray(w_v)[:, kvs], D_T),
            "wlk": _tile_w_fm(wlqk[:, ls], LHL, D_T),
            "wlv": _tile_w_tm(np.asarray(w_lv)[:, ls], D_T),
            "wo": _tile_w_out(np.asarray(w_o)[qs, :]),
            "wlo": _tile_w_out(np.asarray(w_lo)[ls, :]),
            "cosT": cosT,
            "sinTs": sinTs,
            "maskd": maskd,
        })
    return in_maps


def kernel(hidden_states, w_q, w_k, w_v, w_o, w_lq, w_lk, w_lv, w_lo):
    nc = _get_program()
    in_maps = build_in_maps(hidden_states, w_q, w_k, w_v, w_o,
                            w_lq, w_lk, w_lv, w_lo)
    res = run_bass_kernel_spmd(nc, in_maps, list(range(NCORES))).results

    out = np.zeros((B, S, D), dtype=np.float32)
    for c in range(NCORES):
        b = c // TP
        out[b] += res[c]["out"]
    return out


# revision 22
# speedup vs baseline: 1.0114x; 1.0067x over previous
"""Trainium2 Bass kernel for LlamaMultiheadLatentAttention.

Contract: kernel(**inputs) takes FULL fp32 inputs (as produced by
reference.setup_inputs) and returns the FULL fp32 output [2, 1024, 4096].

Sharding (8 cores, no collectives): core c handles batch b = c//4 and
head-group g = c%4 (8 query heads, 2 kv heads, 8 latent heads). q/k/v and
latent projections are column-sharded per head-group; o_proj/latent_o_proj
are row-sharded, so each core emits a partial output sum and the host adds
the 4 partials per batch (the output all-reduce happens at unshard time).

Key design points (baseline 1.01ms -> ~0.75ms):
  - all matmul operands bf16 (fp8 was measured numerically infeasible for
    the 2e-2 gate: even w_v/w_lv alone in e4m3 gives rel=2.8e-2).
  - w_lq @ w_lk composed on the host into one weight: the lq GEMM
    disappears (-27us TensorE/core) and numerics slightly improve.
  - feature-major activations; attention scores computed transposed
    S^T[j,i] so softmax/PV need no transposes.
  - causal trimming by shortened moving streams: a diagonal j-block's
    score/PV/denominator matmuls only stream queries i >= 128*jb into a
    nested PSUM sub-range (the per-element has_written bits make
    overwrite-then-accumulate across nested ranges correct). -31us PE,
    -25% exp elements, masks shrink to one [128,128] multiply per block.
  - softmax denominator via ones-stationary matmuls -> [1,512]
    reciprocal_approx_fast -> gpsimd partition_broadcast (off the
    DVE/ACT critical path; v1's [1,512] InstReciprocal cost 3.3us each).
  - software-pipelined per-head emission: scores+exp early, next head's
    q-projection between, PV after, so ScalarE's ~110us of exp hides
    entirely under projection matmuls and HAM stays at K=8/8 for the
    whole kernel body.
  - pool/DMA orchestration: weight-stream pool allocated before the B2
    weight pool (new-pool writes wait on the WHOLE previous pool's
    release, which otherwise serializes the first B1 weight DMA behind
    all of B2); warmup-result DMA on the gpsimd queue (on the sync queue
    it head-of-line-blocks every input DMA); startup DMAs chunked and
    first chunks split across queues (one queue sustains ~1/16 of HBM
    BW); HAM warmup matmuls cover the initial DMA latency.
  - all PSUM->SBUF copies pinned to DVE (nc.any copies land on ScalarE
    which is both slow for copies and busy with exp).
"""

import numpy as np
import ml_dtypes

import concourse.bass as bass
import concourse.mybir as mybir
import concourse.tile as tile
from concourse import bacc
from concourse.bass_utils import run_bass_kernel_spmd

BF16 = ml_dtypes.bfloat16

B, S, D = 2, 1024, 4096
H, KVH, HD = 32, 8, 128
GROUPS = H // KVH
LAT, LH = 1024, 32
THETA = 10000.0
SCALE = 1.0 / float(np.sqrt(HD))

NCORES = 8
TP = 4                 # head-group shards
HL = H // TP           # 8 local q heads
KVL = KVH // TP        # 2 local kv heads
LHL = LH // TP         # 8 local latent heads

f32 = mybir.dt.float32
bf16 = mybir.dt.bfloat16

D_T = D // 128         # 32 k-tiles over model dim
S_T = S // 128         # 8 token tiles of 128
IB = 2                 # token blocks of 512
XT_CHUNK = 4           # kt per xt DMA chunk (8 chunks)


def _build_program():
    nc = bacc.Bacc("TRN2", target_bir_lowering=False, debug=False)

    xt_d = nc.dram_tensor("xt", [128, D_T, S], bf16, kind="ExternalInput")
    wq_d = nc.dram_tensor("wq", [HL, 128, D_T, 128], bf16, kind="ExternalInput")
    wk_d = nc.dram_tensor("wk", [KVL, 128, D_T, 128], bf16, kind="ExternalInput")
    wv_d = nc.dram_tensor("wv", [128, D_T, KVL * HD], bf16, kind="ExternalInput")
    wlk_d = nc.dram_tensor("wlk", [LHL, 128, D_T, 128], bf16, kind="ExternalInput")
    wlv_d = nc.dram_tensor("wlv", [128, D_T, LHL * HD], bf16, kind="ExternalInput")
    wo_d = nc.dram_tensor("wo", [8, 128, HL, 512], bf16, kind="ExternalInput")
    wlo_d = nc.dram_tensor("wlo", [8, 128, LHL, 512], bf16, kind="ExternalInput")
    cos_d = nc.dram_tensor("cosT", [HD, S], bf16, kind="ExternalInput")
    sin_d = nc.dram_tensor("sinTs", [HD, S], bf16, kind="ExternalInput")
    mask_d = nc.dram_tensor("maskd", [128, 128], bf16, kind="ExternalInput")
    out_d = nc.dram_tensor("out", [S, D], f32, kind="ExternalOutput")
    warm_d = nc.dram_tensor("warm", [128, 512], f32, kind="ExternalOutput")

    out_ap = out_d.ap().rearrange("(tt p) d -> p tt d", p=128)

    with tile.TileContext(nc) as tc:
        with tc.tile_pool(name="const", bufs=1) as constp, \
             tc.tile_pool(name="acts", bufs=1) as acts:

            ones = constp.tile([128, 512], bf16, tag="ones")
            nc.vector.memset(ones[:], 1.0)
            cosT = constp.tile([HD, S], bf16, tag="cosT")
            sinTs = constp.tile([HD, S], bf16, tag="sinTs")
            maskd = constp.tile([128, 128], bf16, tag="maskd")
            # const DMAs on the gpsimd queue: the sync queue is the critical
            # path for the first projection operands
            nc.gpsimd.dma_start(cosT[:], cos_d.ap())
            nc.gpsimd.dma_start(sinTs[:], sin_d.ap())
            nc.gpsimd.dma_start(maskd[:], mask_d.ap())

            # persistent activations (bf16)
            kT = acts.tile([128, KVL, S], bf16, tag="kT")
            lkT = acts.tile([128, LHL, S], bf16, tag="lkT")
            v_sb = acts.tile([128, S_T, KVL * HD], bf16, tag="v")
            lv_sb = acts.tile([128, S_T, LHL * HD], bf16, tag="lv")
            attnT = acts.tile([128, HL, S], bf16, tag="attnT")
            latT = acts.tile([128, LHL, S], bf16, tag="latT")

            # ---- phase W: HAM warmup during initial DMA wait ----
            with tc.tile_pool(name="warm", bufs=1) as warmp, \
                 tc.tile_pool(name="ps_w", bufs=1, space="PSUM") as psw:
                ps_wt = psw.tile([128, 512], f32, tag="ps_w")
                NWARM = 12
                for i in range(NWARM):
                    nc.tensor.matmul(ps_wt[:], ones[:, 0:128], ones[:],
                                     start=(i == 0), stop=(i == NWARM - 1))
                wsb = warmp.tile([128, 512], f32, tag="wsb")
                nc.vector.tensor_copy(wsb[:], ps_wt[:])
                nc.gpsimd.dma_start(warm_d.ap(), wsb[:])

            with tc.tile_pool(name="xt", bufs=1) as xtp, \
                 tc.tile_pool(name="wstr", bufs=2) as wstr:
                xt = xtp.tile([128, D_T, S], bf16, tag="xt")

                # ---- phase B2: token-major projections v, lv ----
                # kt-outer passes (compute paces with the chunked DMAs),
                # capped at 6 PSUM banks so the next phase can start while
                # the last pass drains.
                with tc.tile_pool(name="wvlv", bufs=1) as wvp, \
                     tc.tile_pool(name="ps_b2", bufs=6, space="PSUM") as psb2:
                    # allocation order matters: the stack allocator reclaims
                    # top-down, so wv (whose readers finish 40us before the
                    # wlv half-1 readers) must sit on top for the next
                    # phase's weight-stream pool to reuse its range early.
                    wlv_sb = [wvp.tile([128, D_T, 512], bf16, tag="wlvh",
                                       name=f"wlvh_{h}") for h in range(2)]
                    wv_sb = wvp.tile([128, D_T, KVL * HD], bf16, tag="wv")
                    # interleaved startup DMAs: xt / wv / wlv half0 by chunk.
                    # Only the first xt chunk is split per-kt: one queue only
                    # sustains ~1/16 of HBM BW, so splitting gets the first
                    # matmul's operands in flight on parallel queues, while
                    # keeping the total dma_start count low (sync-engine
                    # issue is ~0.6us per dma_start).
                    nc.sync.dma_start(xt[:, 0:1, 0:512],
                                      xt_d.ap()[:, 0:1, 0:512])
                    nc.sync.dma_start(wv_sb[:, 0:1, :], wv_d.ap()[:, 0:1, :])
                    nc.sync.dma_start(wlv_sb[0][:, 0:1, :],
                                      wlv_d.ap()[:, 0:1, 0:512])
                    nc.sync.dma_start(xt[:, 0:1, 512:1024],
                                      xt_d.ap()[:, 0:1, 512:1024])
                    for kt in range(1, XT_CHUNK):
                        ks = bass.ds(kt, 1)
                        nc.sync.dma_start(xt[:, ks, :], xt_d.ap()[:, ks, :])
                        nc.sync.dma_start(wv_sb[:, ks, :], wv_d.ap()[:, ks, :])
                        nc.sync.dma_start(wlv_sb[0][:, ks, :],
                                          wlv_d.ap()[:, ks, 0:512])
                    for c in range(1, D_T // XT_CHUNK):
                        # split each later chunk in two so transfers ride two
                        # DMA queues (halves the per-chunk arrival latency)
                        for hh in range(2):
                            cs = bass.ds(c * XT_CHUNK + hh * (XT_CHUNK // 2),
                                         XT_CHUNK // 2)
                            nc.sync.dma_start(xt[:, cs, :],
                                              xt_d.ap()[:, cs, :])
                            nc.sync.dma_start(wv_sb[:, cs, :],
                                              wv_d.ap()[:, cs, :])
                            nc.sync.dma_start(wlv_sb[0][:, cs, :],
                                              wlv_d.ap()[:, cs, 0:512])

                    # half 0: lv + v together (shared stationary xt tiles)
                    for tts in ([0, 1, 2], [3, 4, 5], [6, 7]):
                        ps_lv = {tt: psb2.tile([128, 512], f32, tag="ps_b2",
                                               name=f"ps_lv0_{tt}")
                                 for tt in tts}
                        ps_v = {tt: psb2.tile([128, KVL * HD], f32,
                                              tag="ps_b2", name=f"ps_v_{tt}")
                                for tt in tts}
                        for kt in range(D_T):
                            st, sp = kt == 0, kt == D_T - 1
                            for tt in tts:
                                lhs = xt[:, kt, bass.ts(tt, 128)]
                                nc.tensor.matmul(ps_lv[tt][:], lhs,
                                                 wlv_sb[0][:, kt, :],
                                                 start=st, stop=sp)
                                nc.tensor.matmul(ps_v[tt][:], lhs,
                                                 wv_sb[:, kt, :],
                                                 start=st, stop=sp)
                        for tt in tts:
                            nc.vector.tensor_copy(lv_sb[:, tt, 0:512],
                                                  ps_lv[tt][:])
                            nc.vector.tensor_copy(v_sb[:, tt, :], ps_v[tt][:])

                    # half-1 DMA (chunked; slot reuses only after pass 2 reads)
                    for c in range(D_T // XT_CHUNK):
                        cs = bass.ds(c * XT_CHUNK, XT_CHUNK)
                        nc.sync.dma_start(wlv_sb[1][:, cs, :],
                                          wlv_d.ap()[:, cs, 512:1024])
                    # half 1 of lv
                    for tts in ([0, 1, 2, 3, 4, 5], [6, 7]):
                        ps_lv = {tt: psb2.tile([128, 512], f32, tag="ps_b2",
                                               name=f"ps_lv1_{tt}")
                                 for tt in tts}
                        for kt in range(D_T):
                            st, sp = kt == 0, kt == D_T - 1
                            for tt in tts:
                                nc.tensor.matmul(ps_lv[tt][:],
                                                 xt[:, kt, bass.ts(tt, 128)],
                                                 wlv_sb[1][:, kt, :],
                                                 start=st, stop=sp)
                        for tt in tts:
                            nc.vector.tensor_copy(lv_sb[:, tt, 512:1024],
                                                  ps_lv[tt][:])

                # ---- phases B1 + C (software-pipelined per head) ----
                with tc.tile_pool(name="qq", bufs=2) as qqp, \
                     tc.tile_pool(name="pp", bufs=14) as pp, \
                     tc.tile_pool(name="f32t", bufs=3) as f32t, \
                     tc.tile_pool(name="dn", bufs=2) as dn, \
                     tc.tile_pool(name="dnb", bufs=2) as dnb, \
                     tc.tile_pool(name="ps_s", bufs=3, space="PSUM") as pss_, \
                     tc.tile_pool(name="ps_o", bufs=2, space="PSUM") as pso_, \
                     tc.tile_pool(name="ps_d", bufs=1, space="PSUM") as psd_, \
                     tc.tile_pool(name="ps_b1", bufs=2, space="PSUM") as psb1:

                    def rope_to(dst, ps, ib):
                        sl = bass.ts(ib, 512)
                        rt = f32t.tile([128, 512], f32, tag="f32t")
                        qc = f32t.tile([128, 512], f32, tag="f32t")
                        nc.vector.tensor_tensor(
                            rt[0:64, :], ps[64:128, :], sinTs[0:64, sl],
                            mybir.AluOpType.mult)
                        nc.vector.tensor_tensor(
                            rt[64:128, :], ps[0:64, :], sinTs[64:128, sl],
                            mybir.AluOpType.mult)
                        nc.vector.tensor_tensor(
                            qc[:], ps[:], cosT[:, sl], mybir.AluOpType.mult)
                        nc.vector.tensor_add(dst, qc[:], rt[:])

                    def proj_dma(w_dram, nt):
                        wt = wstr.tile([128, D_T, 128], bf16, tag="w")
                        nc.sync.dma_start(wt[:], w_dram.ap()[nt])
                        return wt

                    def proj_half(wt, key, dst, ib):
                        # dst ib-half = rope(wt.T @ xt), one PSUM bank
                        ps = psb1.tile([128, 512], f32, tag="ps_b1",
                                       name=f"ps_b1_{key}_{ib}")
                        for kt in range(D_T):
                            nc.tensor.matmul(
                                ps[:], wt[:, kt, :],
                                xt[:, kt, bass.ts(ib, 512)],
                                start=(kt == 0), stop=(kt == D_T - 1))
                        rope_to(dst[:, bass.ts(ib, 512)], ps[:], ib)

                    def proj_head(w_dram, nt, dst, key):
                        wt = proj_dma(w_dram, nt)
                        for ib in range(IB):
                            proj_half(wt, f"{key}{nt}", dst, ib)

                    # attention unit = (head dst, ib block); three emission
                    # stages so exp latency hides under projection matmuls
                    def u_off(u, jb):
                        # diagonal j-blocks only need queries i >= 128*jb:
                        # use a shorter moving stream into a nested PSUM
                        # sub-range (later matmuls overwrite-or-accumulate
                        # per-element via the has_written bits).
                        r = jb - 4 * u["ib"]
                        return 128 * r if r >= 0 else 0

                    def u_scores(u):
                        qsrc, ksrc, ib = u["qsrc"], u["ksrc"], u["ib"]
                        for jb in range(u["njb"]):
                            off = u_off(u, jb)
                            sl = bass.ds(off, 512 - off)
                            ps_s = pss_.tile([128, 512], f32, tag="ps_s")
                            nc.tensor.matmul(
                                ps_s[:, sl], ksrc[:, bass.ts(jb, 128)],
                                qsrc[:, bass.ds(ib * 512 + off, 512 - off)],
                                start=True, stop=True)
                            pt = pp.tile([128, 512], bf16, tag="pt")
                            nc.scalar.activation(
                                pt[:, sl], ps_s[:, sl],
                                mybir.ActivationFunctionType.Exp,
                                scale=SCALE)
                            if jb - 4 * ib >= 0:
                                # causal mask on the first 128 cols (j > i)
                                nc.vector.tensor_tensor(
                                    pt[:, bass.ds(off, 128)],
                                    pt[:, bass.ds(off, 128)], maskd[:],
                                    mybir.AluOpType.mult)
                            u["pts"].append(pt)

                    def u_den(u):
                        # softmax denominator: ones-matmuls -> [1,512]
                        # -> fast reciprocal -> gpsimd broadcast
                        ps_d = psd_.tile([1, 512], f32, tag="ps_d")
                        for jb in range(u["njb"]):
                            off = u_off(u, jb)
                            sl = bass.ds(off, 512 - off)
                            nc.tensor.matmul(
                                ps_d[:, sl], ones[:, 0:1], u["pts"][jb][:, sl],
                                start=(jb == 0), stop=(jb == u["njb"] - 1))
                        rec = dn.tile([1, 512], f32, tag="rec")
                        nc.vector.reciprocal_approx_fast(rec[:], ps_d[:])
                        recb = dnb.tile([128, 512], f32, tag="recb")
                        nc.gpsimd.partition_broadcast(recb[:], rec[:])
                        u["recb"] = recb

                    def u_pv(u):
                        vsrc, vofs, ib = u["vsrc"], u["vofs"], u["ib"]
                        ps_o = pso_.tile([128, 512], f32, tag="ps_o")
                        for jb in range(u["njb"]):
                            off = u_off(u, jb)
                            sl = bass.ds(off, 512 - off)
                            nc.tensor.matmul(
                                ps_o[:, sl], vsrc[:, jb, vofs],
                                u["pts"][jb][:, sl],
                                start=(jb == 0), stop=(jb == u["njb"] - 1))
                        nc.vector.tensor_tensor(
                            u["dst"][:, bass.ts(ib, 512)], ps_o[:],
                            u["recb"][:], mybir.AluOpType.mult)

                    def mk_units(h, qsrc):
                        out = []
                        for latent in (False, True):
                            for ib in range(IB):
                                out.append({
                                    "qsrc": qsrc, "ib": ib,
                                    "njb": 4 * (ib + 1), "pts": [],
                                    "ksrc": (lkT[:, h, :] if latent
                                             else kT[:, h // GROUPS, :]),
                                    "vsrc": lv_sb if latent else v_sb,
                                    "vofs": bass.ts(h if latent
                                                    else h // GROUPS, HD),
                                    "dst": (latT[:, h, :] if latent
                                            else attnT[:, h, :]),
                                })
                        return out

                    # B1-k: kv heads (feature-major, roped)
                    for nt in range(KVL):
                        proj_head(wk_d, nt, kT[:, nt, :], "k")
                    # q head 0 early (its weight DMA slots are free here)
                    qq = qqp.tile([128, S], bf16, tag="qq")
                    proj_head(wq_d, 0, qq[:], "q")
                    # B1-lk: latent-key heads via composed w_lq@w_lk
                    for nt in range(LHL):
                        proj_head(wlk_d, nt, lkT[:, nt, :], "lk")

                    # pipelined heads: for head h emit scores/exp early,
                    # weave next head's projection between, PV after.
                    for h in range(HL):
                        A0, A1, L0, L1 = units = mk_units(h, qq)
                        if h + 1 < HL:
                            qq = qqp.tile([128, S], bf16, tag="qq")
                            wt_next = proj_dma(wq_d, h + 1)
                        u_scores(A0)
                        u_scores(A1)
                        u_scores(L0)
                        u_den(A0)
                        u_scores(L1)
                        u_den(A1)
                        if h + 1 < HL:
                            proj_half(wt_next, f"q{h+1}", qq[:], 0)
                        u_den(L0)
                        u_pv(A0)
                        u_pv(A1)
                        if h + 1 < HL:
                            proj_half(wt_next, f"q{h+1}", qq[:], 1)
                        u_den(L1)
                        u_pv(L0)
                        u_pv(L1)

            # ---- phase D: output projections (row-sharded, partial sum) ----
            with tc.tile_pool(name="wop", bufs=2) as wop, \
                 tc.tile_pool(name="ost", bufs=4) as ost, \
                 tc.tile_pool(name="ps_f", bufs=4, space="PSUM") as psf:
                for np_ in range(4):       # pairs of 512-wide col blocks
                    wo2 = wop.tile([128, HL, 1024], bf16, tag="wo2")
                    wlo2 = wop.tile([128, LHL, 1024], bf16, tag="wlo2")
                    for u in range(2):
                        for h in range(HL):     # per-head chunks: D's first
                            nc.sync.dma_start(  # MMs start after ~0.7us
                                wo2[:, h, bass.ts(u, 512)],
                                wo_d.ap()[2 * np_ + u][:, h, :])
                        for h in range(LHL):
                            nc.sync.dma_start(
                                wlo2[:, h, bass.ts(u, 512)],
                                wlo_d.ap()[2 * np_ + u][:, h, :])
                    for tt in range(S_T):
                        ps0 = psf.tile([128, 512], f32, tag="ps_f")
                        ps1 = psf.tile([128, 512], f32, tag="ps_f")
                        for h in range(HL):
                            lhs = attnT[:, h, bass.ts(tt, 128)]
                            nc.tensor.matmul(ps0[:], lhs,
                                             wo2[:, h, 0:512],
                                             start=(h == 0), stop=False)
                            nc.tensor.matmul(ps1[:], lhs,
                                             wo2[:, h, 512:1024],
                                             start=(h == 0), stop=False)
                        for h in range(LHL):
                            lhs = latT[:, h, bass.ts(tt, 128)]
                            nc.tensor.matmul(ps0[:], lhs,
                                             wlo2[:, h, 0:512],
                                             start=False, stop=(h == LHL - 1))
                            nc.tensor.matmul(ps1[:], lhs,
                                             wlo2[:, h, 512:1024],
                                             start=False, stop=(h == LHL - 1))
                        for u, ps in enumerate((ps0, ps1)):
                            ot = ost.tile([128, 512], f32, tag="ot")
                            nc.vector.tensor_copy(ot[:], ps[:])
                            nc.sync.dma_start(
                                out_ap[:, tt, bass.ds(
                                    (2 * np_ + u) * 512, 512)],
                                ot[:])

    nc.compile()
    return nc


_NC = None


def _get_program():
    global _NC
    if _NC is None:
        _NC = _build_program()
    return _NC


def _rope_tables():
    inv_freq = 1.0 / (THETA ** (np.arange(0, HD, 2, dtype=np.float32) / HD))
    t = np.arange(S, dtype=np.float32)
    freqs = np.outer(t, inv_freq)                       # [S, 64]
    emb = np.concatenate([freqs, freqs], axis=-1)       # [S, HD]
    cosT = np.cos(emb).T.astype(BF16).copy()            # [HD, S]
    sinT = np.sin(emb).T.astype(np.float32)
    sinTs = np.concatenate([-sinT[:HD // 2], sinT[HD // 2:]], 0).astype(
        BF16).copy()
    return cosT, sinTs


def _mask_patterns():
    # maskd[p, i] = 1.0 iff p <= i (diagonal 128x128 causal block)
    p = np.arange(128)[:, None]
    i = np.arange(128)[None, :]
    return (p <= i).astype(BF16)


def _tile_w_fm(w, n_tiles, kt):
    # [K, n_tiles*128] -> [n_tiles, 128(p of K), kt, 128]
    K, N = w.shape
    assert K == kt * 128 and N == n_tiles * 128
    return np.ascontiguousarray(
        w.reshape(kt, 128, n_tiles, 128).transpose(2, 1, 0, 3)).astype(BF16)


def _tile_w_tm(w, kt):
    # [K, N] -> [128(p of K), kt, N]
    K, N = w.shape
    assert K == kt * 128
    return np.ascontiguousarray(
        w.reshape(kt, 128, N).transpose(1, 0, 2)).astype(BF16)


def _tile_w_out(w):
    # [1024, D] -> [8(nb), 128(p of rows), 8(h), 512]
    return np.ascontiguousarray(
        w.reshape(8, 128, D // 512, 512).transpose(2, 1, 0, 3)).astype(BF16)


def build_in_maps(hidden_states, w_q, w_k, w_v, w_o, w_lq, w_lk, w_lv, w_lo):
    cosT, sinTs = _rope_tables()
    maskd = _mask_patterns()
    # compose the latent-q/latent-k projections: lk = x @ (w_lq @ w_lk)
    wlqk = np.asarray(w_lq, np.float32) @ np.asarray(w_lk, np.float32)

    in_maps = []
    for c in range(NCORES):
        b, g = divmod(c, TP)
        x = np.asarray(hidden_states[b], dtype=np.float32)       # [S, D]
        xt = np.ascontiguousarray(
            x.T.reshape(D_T, 128, S).transpose(1, 0, 2)).astype(BF16)
        qs = slice(g * HL * HD, (g + 1) * HL * HD)
        kvs = slice(g * KVL * HD, (g + 1) * KVL * HD)
        ls = slice(g * LHL * HD, (g + 1) * LHL * HD)
        in_maps.append({
            "xt": xt,
            "wq": _tile_w_fm(np.asarray(w_q)[:, qs], HL, D_T),
            "wk": _tile_w_fm(np.asarray(w_k)[:, kvs], KVL, D_T),
            "wv": _tile_w_tm(np.asarray(w_v)[:, kvs], D_T),
            "wlk": _tile_w_fm(wlqk[:, ls], LHL, D_T),
            "wlv": _tile_w_tm(np.asarray(w_lv)[:, ls], D_T),
            "wo": _tile_w_out(np.asarray(w_o)[qs, :]),
            "wlo": _tile_w_out(np.asarray(w_lo)[ls, :]),
            "cosT": cosT,
            "sinTs": sinTs,
            "maskd": maskd,
        })
    return in_maps


def kernel(hidden_states, w_q, w_k, w_v, w_o, w_lq, w_lk, w_lv, w_lo):
    nc = _get_program()
    in_maps = build_in_maps(hidden_states, w_q, w_k, w_v, w_o,
                            w_lq, w_lk, w_lv, w_lo)
    res = run_bass_kernel_spmd(nc, in_maps, list(range(NCORES))).results

    out = np.zeros((B, S, D), dtype=np.float32)
    for c in range(NCORES):
        b = c // TP
        out[b] += res[c]["out"]
    return out
